# revision 1
# baseline (speedup 1.0000x reference)
"""DeepSeekMoE layer on 8 Trainium2 NeuronCores.

Strategy: token-parallel (data-parallel) across the 8 cores. Each core
processes 512 of the 4096 tokens: RMSNorm -> router (top-2 of 8, exact
fp32) -> all 8 routed experts (dense, masked by combine weights) + shared
expert. Expert matmuls run in fp32r (TF32-like) at full PE rate with
activations kept feature-major (transposed) so every matmul contracts on
the partition dim without extra weight transposes; weights are pre-packed
on the host into the exact SBUF tile layout.

Output is produced transposed per core (D-major) and untransposed on host
during the gather step.
"""
import sys

sys.path.insert(0, "/opt/trn_rl_repo")

import numpy as np
import concourse.bass as bass
import concourse.mybir as mybir
from concourse.masks import make_identity
from concourse.tile import TileContext, ScopedClock
from concourse.bass_utils import run_bass_kernel_spmd

fp32 = mybir.dt.float32
fp32r = mybir.dt.float32r
fp16 = mybir.dt.float16
i32 = mybir.dt.int32

# dtype used for the expert matmuls (weights + activations).
#  - fp32r: TF32-like, ~1.4e-4 matmul rel err
#  - fp16: ~5e-4 rel err, but 2-byte: half the weight DMA and pipelined
#    weight loads on the PE
import os
MM_DT_NAME = os.environ.get("MOE_MM_DT", "fp32r")
AF = mybir.ActivationFunctionType
ALU = mybir.AluOpType
AX = mybir.AxisListType

# problem dims (hardcoded per contract)
B, T, D, F, E, K = 4, 1024, 1024, 512, 8, 2
N_CORES = 8
N = B * T              # 4096 tokens
C = N // N_CORES       # 512 tokens per core
CT = C // 128          # token tiles per core (4)
DT = D // 128          # d tiles (8)
FT = F // 128          # f tiles (4)
GATE_MAX = 30.0
LIN_MIN, LIN_MAX = -100.0, 100.0
EPS_RMS = 1e-6

# Exact clip ops (min(g,30), clip(u,+-100)). The reference applies them;
# with the given weight scales the bounds are never active, but they are
# cheap, so keep them on for exactness.
APPLY_CLIPS = True

# ---------------------------------------------------------------------------
# Workaround: this container's walrus build only accepts ONE semaphore wait
# per instruction ("Too many sync wait commands"). Split excess waits onto
# same-engine NoOp/Drain instructions.
# ---------------------------------------------------------------------------
MAX_WAITS = 1


class PatchedTileContext(TileContext):
    def _drain_and_barrier(self, tick_clock, wait_clock):
        drain_inst = self.nc.sync.drain()
        wait_clock.add_sem_waits(
            drain_inst.ins, ScopedClock({None: tick_clock.global_clock})
        )
        si = drain_inst.ins.sync_info
        waits = list(si.on_wait) if si is not None else []
        if len(waits) > MAX_WAITS:
            drain_inst.ins.sync_info.on_wait.clear()
            drain_inst.ins.sync_info.on_wait.extend(waits[:MAX_WAITS])
            for i in range(MAX_WAITS, len(waits), MAX_WAITS):
                extra = self.nc.sync.drain()
                extra.ins.sync_info = mybir.SyncInfo(
                    on_wait=list(waits[i : i + MAX_WAITS]), on_update=[]
                )
        self.nc.all_engine_barrier()
        assert self.sems is not None
        popped = self.nc._tile_sem_poison_stack.pop()
        assert popped is self._sem_poison
        self.nc.clear_and_free_semaphores(list(self.sems.allocated().values()))
        self.nc.all_engine_barrier()


def fix_excess_waits(nc, max_waits=MAX_WAITS):
    n_fixed = 0
    counter = [0]
    for f in nc.m.functions:
        for bb in f.blocks:
            il = bb.instructions
            new_list = []
            for inst in il:
                si = getattr(inst, "sync_info", None)
                waits = list(si.on_wait) if si is not None else []
                if len(waits) > max_waits:
                    n_fixed += 1
                    keep = waits[:max_waits]
                    rest = waits[max_waits:]
                    si.on_wait.clear()
                    si.on_wait.extend(keep)
                    for i in range(0, len(rest), max_waits):
                        counter[0] += 1
                        nop = mybir.InstNoOp(
                            name=f"I-waitfix-{counter[0]}", ins=[], outs=[]
                        )
                        nop.engine = inst.engine
                        nop.sync_info = mybir.SyncInfo(
                            on_wait=list(rest[i : i + max_waits]), on_update=[]
                        )
                        new_list.append(nop)
                new_list.append(inst)
            if len(new_list) != len(il):
                il.clear()
                il.extend(new_list)
    return n_fixed


# ---------------------------------------------------------------------------
# Device program (SPMD; identical on all 8 cores)
# ---------------------------------------------------------------------------

def build_nc(repeat=1, const_weights=None):
    mmdt = {"fp32r": fp32r, "fp16": fp16}[MM_DT_NAME]
    nc = bass.Bass("TRN2", target_bir_lowering=False, debug=False,
                   num_devices=N_CORES)

    def _wtensor(name, shape, dtype):
        if const_weights is not None and name in const_weights:
            data = np.ascontiguousarray(const_weights[name])
            nc.inline_tensor(data, name=name)
            mls = nc.lookup_mls(name)
            mls.dtype = dtype
            from concourse.bass_types import DRamTensorHandle as _DH
            return _DH(name, list(data.shape), dtype).ap()
        return nc.dram_tensor(name, shape, dtype, kind="ExternalInput").ap()

    x_d = nc.dram_tensor("x", [CT, 128, D], fp32, kind="ExternalInput").ap()
    vis_d = nc.dram_tensor("vis", [CT, 128, 1], i32, kind="ExternalInput").ap()
    rmsw_d = _wtensor("rmsw", [D], fp32)
    rwT_d = _wtensor("rwT", [128, DT, E], fp32)
    bias0_d = _wtensor("bias0", [E], fp32)
    bias1_d = _wtensor("bias1", [E], fp32)
    # routed expert weights, pre-packed: (E, 128, nk, out_dim)
    wgT_d = _wtensor("wgT", [E, 128, DT, F], mmdt)
    wuT_d = _wtensor("wuT", [E, 128, DT, F], mmdt)
    wdT_d = _wtensor("wdT", [E, 128, FT, D], mmdt)
    shgT_d = _wtensor("shgT", [128, DT, F], mmdt)
    shuT_d = _wtensor("shuT", [128, DT, F], mmdt)
    shdT_d = _wtensor("shdT", [128, FT, D], mmdt)

    outT_d = nc.dram_tensor("outT", [DT, 128, C], fp32, kind="ExternalOutput").ap()
    comb_dram = nc.dram_tensor("comb_scratch", [E, C], fp32).ap()

    with PatchedTileContext(nc) as tc:
        with (
            tc.tile_pool(name="const", bufs=1) as const,
            tc.tile_pool(name="xin", bufs=2) as xin,
            tc.tile_pool(name="hbuf", bufs=2) as hbuf,
            tc.tile_pool(name="persist", bufs=1) as persist,
            tc.tile_pool(name="router", bufs=4) as router,
            tc.tile_pool(name="combp", bufs=1) as combp,
            tc.tile_pool(name="wpool", bufs=2) as wpool,
            tc.tile_pool(name="wupool", bufs=1) as wupool,
            tc.tile_pool(name="wdpool", bufs=1) as wdpool,
            tc.tile_pool(name="act", bufs=2) as actp,
            tc.tile_pool(name="a2pool", bufs=2) as a2pool,
            tc.tile_pool(name="pst", bufs=2, space="PSUM") as pst,
            tc.tile_pool(name="psgu", bufs=2, space="PSUM") as psgu,
            tc.tile_pool(name="psy", bufs=2, space="PSUM") as psy,
        ):
            ident = const.tile([128, 128], fp32)
            make_identity(nc, ident[:])
            eps_t = const.tile([128, 1], fp32)
            nc.vector.memset(eps_t[:], EPS_RMS)
            rmsw_bc = const.tile([128, D], fp32)
            nc.gpsimd.dma_start(
                out=rmsw_bc[:],
                in_=bass.AP(tensor=rmsw_d.tensor, offset=rmsw_d.offset,
                            ap=[[0, 128]] + list(rmsw_d.ap)),
            )
            base_bc = const.tile([128, E], fp32)
            nc.gpsimd.dma_start(
                out=base_bc[:],
                in_=bass.AP(tensor=bias0_d.tensor, offset=bias0_d.offset,
                            ap=[[0, 128]] + list(bias0_d.ap)),
            )
            delta_bc = const.tile([128, E], fp32)
            nc.gpsimd.dma_start(
                out=delta_bc[:],
                in_=bass.AP(tensor=bias1_d.tensor, offset=bias1_d.offset,
                            ap=[[0, 128]] + list(bias1_d.ap)),
            )
            rwT = const.tile([128, DT, E], fp32)
            nc.gpsimd.dma_start(out=rwT[:], in_=rwT_d[:])

            hT = persist.tile([128, DT, C], fp32)      # h transposed (exact)
            hTr = persist.tile([128, DT, C], mmdt)    # fp32r copy for experts
            comb_bc = persist.tile([128, E, C], fp32)  # combine weights bcast
            outT = persist.tile([128, DT, C], fp32)    # output accumulator

            for r in range(repeat):
                # ---- RMSNorm + transpose h
                for tt in range(CT):
                    xt = xin.tile([128, D], fp32)
                    nc.gpsimd.dma_start(out=xt[:], in_=x_d[tt])
                    sq = hbuf.tile([128, D], fp32, tag="h")
                    var = router.tile([128, 1], fp32)
                    nc.scalar.activation(sq[:], xt[:], AF.Square,
                                         accum_out=var[:])
                    s = router.tile([128, 1], fp32)
                    nc.scalar.activation(s[:], var[:], AF.Sqrt,
                                         scale=1.0 / D, bias=eps_t[:])
                    rstd = router.tile([128, 1], fp32)
                    nc.vector.reciprocal(rstd[:], s[:])
                    ht = hbuf.tile([128, D], fp32, tag="h")
                    nc.vector.scalar_tensor_tensor(
                        ht[:], xt[:], rstd[:], rmsw_bc[:],
                        op0=ALU.mult, op1=ALU.mult)
                    for dt in range(DT):
                        tp = pst.tile([128, 128], fp32, tag="ps")
                        nc.tensor.transpose(
                            tp[:], ht[:, dt * 128:(dt + 1) * 128], ident[:])
                        nc.scalar.copy(hT[:, dt, tt * 128:(tt + 1) * 128], tp[:])
                        nc.vector.tensor_copy(
                            hTr[:, dt, tt * 128:(tt + 1) * 128], tp[:])

                # ---- router: z, aff, top-2, comb (token-major, exact fp32)
                combT = combp.tile([128, C], fp32)
                for tt in range(CT):
                    zp = pst.tile([128, E], fp32, tag="ps")
                    for dt in range(DT):
                        nc.tensor.matmul(
                            zp[:], hT[:, dt, tt * 128:(tt + 1) * 128],
                            rwT[:, dt, :], start=(dt == 0), stop=(dt == DT - 1))
                    aff = router.tile([128, E], fp32)
                    nc.scalar.activation(aff[:], zp[:], AF.Sigmoid)
                    vist = router.tile([128, 1], i32)
                    nc.gpsimd.dma_start(out=vist[:], in_=vis_d[tt])
                    visf = router.tile([128, 1], fp32)
                    nc.vector.tensor_copy(visf[:], vist[:])
                    biased = router.tile([128, E], fp32)
                    nc.vector.tensor_add(biased[:], aff[:], base_bc[:])
                    nc.vector.scalar_tensor_tensor(
                        biased[:], delta_bc[:], visf[:], biased[:],
                        op0=ALU.mult, op1=ALU.add)
                    mx1 = router.tile([128, 1], fp32)
                    nc.vector.tensor_reduce(mx1[:], biased[:], AX.X, ALU.max)
                    msk1 = router.tile([128, E], fp32)
                    nc.vector.tensor_scalar(msk1[:], biased[:], mx1[:], None,
                                            ALU.is_ge)
                    biased2 = router.tile([128, E], fp32)
                    nc.vector.scalar_tensor_tensor(
                        biased2[:], msk1[:], -1e9, biased[:],
                        op0=ALU.mult, op1=ALU.add)
                    mx2 = router.tile([128, 1], fp32)
                    nc.vector.tensor_reduce(mx2[:], biased2[:], AX.X, ALU.max)
                    msk2 = router.tile([128, E], fp32)
                    nc.vector.tensor_scalar(msk2[:], biased2[:], mx2[:], None,
                                            ALU.is_ge)
                    tmp = router.tile([128, E], fp32)
                    g1 = router.tile([128, 1], fp32)
                    nc.vector.tensor_tensor(tmp[:], msk1[:], aff[:], ALU.mult)
                    nc.vector.tensor_reduce(g1[:], tmp[:], AX.X, ALU.add)
                    g2 = router.tile([128, 1], fp32)
                    nc.vector.tensor_tensor(tmp[:], msk2[:], aff[:], ALU.mult)
                    nc.vector.tensor_reduce(g2[:], tmp[:], AX.X, ALU.add)
                    den = router.tile([128, 1], fp32)
                    nc.vector.tensor_add(den[:], g1[:], g2[:])
                    nc.vector.tensor_scalar_add(den[:], den[:], 1e-12)
                    inv = router.tile([128, 1], fp32)
                    nc.vector.reciprocal(inv[:], den[:])
                    comb = router.tile([128, E], fp32)
                    nc.vector.tensor_scalar(comb[:], msk1[:], g1[:], None,
                                            ALU.mult)
                    nc.vector.scalar_tensor_tensor(
                        comb[:], msk2[:], g2[:], comb[:],
                        op0=ALU.mult, op1=ALU.add)
                    nc.vector.tensor_scalar(comb[:], comb[:], inv[:], None,
                                            ALU.mult)
                    # transpose comb (128, E) -> (E, 128) into combT
                    ctp = pst.tile([128, 128], fp32, tag="ps")
                    nc.tensor.transpose(ctp[:E + 0, :], comb[:], ident[:])
                    nc.scalar.copy(combT[:E, tt * 128:(tt + 1) * 128],
                                   ctp[:E, :])
                # broadcast each expert's combine row to 128 partitions
                # (bounce through DRAM: SBUF APs cannot partition-broadcast)
                nc.gpsimd.dma_start(out=comb_dram[:], in_=combT[:E, :])
                for e in range(E):
                    row = comb_dram[e]
                    nc.gpsimd.dma_start(
                        out=comb_bc[:, e, :],
                        in_=bass.AP(tensor=row.tensor, offset=row.offset,
                                    ap=[[0, 128]] + list(row.ap)),
                    )

                # ---- expert pipeline: shared first, then routed experts
                for si in range(E + 1):
                    if si == 0:
                        wg_src, wu_src, wd_src = shgT_d[:], shuT_d[:], shdT_d[:]
                    else:
                        e = si - 1
                        wg_src, wu_src, wd_src = wgT_d[e], wuT_d[e], wdT_d[e]
                    wg_t = wpool.tile([128, DT, F], mmdt, tag="wg")
                    nc.gpsimd.dma_start(out=wg_t[:], in_=wg_src)
                    wu_t = wupool.tile([128, DT, F], mmdt, tag="wu")
                    nc.gpsimd.dma_start(out=wu_t[:], in_=wu_src)
                    wd_t = wdpool.tile([128, FT, D], mmdt, tag="wd")
                    nc.gpsimd.dma_start(out=wd_t[:], in_=wd_src)

                    a2 = a2pool.tile([128, FT, C], mmdt)
                    for ft in range(FT):
                        gp = psgu.tile([128, C], fp32)
                        for dt in range(DT):
                            nc.tensor.matmul(
                                gp[:], wg_t[:, dt, ft * 128:(ft + 1) * 128],
                                hTr[:, dt, :], start=(dt == 0),
                                stop=(dt == DT - 1))
                        sg = actp.tile([128, C], fp32, tag="sg")
                        if APPLY_CLIPS:
                            gm = actp.tile([128, C], fp32, tag="tmp")
                            nc.vector.tensor_scalar_min(gm[:], gp[:], GATE_MAX)
                            nc.scalar.activation(sg[:], gm[:], AF.Silu)
                        else:
                            nc.scalar.activation(sg[:], gp[:], AF.Silu)
                        up = psgu.tile([128, C], fp32)
                        for dt in range(DT):
                            nc.tensor.matmul(
                                up[:], wu_t[:, dt, ft * 128:(ft + 1) * 128],
                                hTr[:, dt, :], start=(dt == 0),
                                stop=(dt == DT - 1))
                        if APPLY_CLIPS:
                            uc = actp.tile([128, C], fp32, tag="tmp")
                            nc.vector.tensor_scalar(uc[:], up[:], LIN_MAX,
                                                    LIN_MIN, ALU.min, ALU.max)
                            usrc = uc
                        else:
                            usrc = up
                        if si == 0:
                            nc.vector.tensor_tensor(a2[:, ft, :], sg[:],
                                                    usrc[:], ALU.mult)
                        else:
                            nc.vector.tensor_tensor(sg[:], sg[:], usrc[:],
                                                    ALU.mult)
                            nc.vector.tensor_tensor(a2[:, ft, :], sg[:],
                                                    comb_bc[:, si - 1, :],
                                                    ALU.mult)
                    for dt in range(DT):
                        yp = psy.tile([128, C], fp32)
                        for ft in range(FT):
                            nc.tensor.matmul(
                                yp[:], wd_t[:, ft, dt * 128:(dt + 1) * 128],
                                a2[:, ft, :], start=(ft == 0),
                                stop=(ft == FT - 1))
                        if si == 0:
                            nc.vector.tensor_copy(outT[:, dt, :], yp[:])
                        else:
                            nc.vector.tensor_tensor(outT[:, dt, :], yp[:],
                                                    outT[:, dt, :], ALU.add)

                # ---- store output (transposed layout; host untransposes)
                for dt in range(DT):
                    nc.gpsimd.dma_start(out=outT_d[dt], in_=outT[:, dt, :])

    fix_excess_waits(nc)
    return nc


def _pack(w):
    """(out_dim, in_dim) weight -> (128, in_dim/128, out_dim) lhsT tiles."""
    out_dim, in_dim = w.shape
    nk = in_dim // 128
    return np.ascontiguousarray(
        w.T.reshape(nk, 128, out_dim).transpose(1, 0, 2))


_CACHE = {}


def _cast(a):
    return a.astype(np.float16) if MM_DT_NAME == "fp16" else a


def _prep(x, is_visual, rms_w, router_w, aux_bias, mod_bias,
          sh_wg, sh_wu, sh_wd, wg, wu, wd):
    xf = np.ascontiguousarray(np.asarray(x, np.float32).reshape(N, D))
    visf = np.asarray(is_visual, np.int32).reshape(N, 1)
    shared = {
        "rmsw": np.asarray(rms_w, np.float32),
        "rwT": _pack(np.asarray(router_w, np.float32)),
        "bias0": np.asarray(aux_bias, np.float32)
        + np.asarray(mod_bias, np.float32)[0],
        "bias1": np.asarray(mod_bias, np.float32)[1]
        - np.asarray(mod_bias, np.float32)[0],
        "wgT": _cast(np.stack([_pack(np.asarray(wg, np.float32)[e]) for e in range(E)])),
        "wuT": _cast(np.stack([_pack(np.asarray(wu, np.float32)[e]) for e in range(E)])),
        "wdT": _cast(np.stack([_pack(np.asarray(wd, np.float32)[e]) for e in range(E)])),
        "shgT": _cast(_pack(np.asarray(sh_wg, np.float32))),
        "shuT": _cast(_pack(np.asarray(sh_wu, np.float32))),
        "shdT": _cast(_pack(np.asarray(sh_wd, np.float32))),
    }
    in_maps = []
    for c in range(N_CORES):
        m = dict(shared)
        m["x"] = xf[c * C:(c + 1) * C].reshape(CT, 128, D)
        m["vis"] = visf[c * C:(c + 1) * C].reshape(CT, 128, 1)
        in_maps.append(m)
    return in_maps


def kernel(**inputs):
    if "nc" not in _CACHE:
        _CACHE["nc"] = build_nc()
    nc = _CACHE["nc"]
    in_maps = _prep(**inputs)
    res = run_bass_kernel_spmd(nc, in_maps, list(range(N_CORES)))
    parts = []
    for c in range(N_CORES):
        outT = res.results[c]["outT"]  # (DT, 128, C)
        parts.append(outT.transpose(2, 0, 1).reshape(C, D))
    return np.concatenate(parts, axis=0).reshape(B, T, D).astype(np.float32)



# revision 3
# speedup vs baseline: 5.6317x; 5.6317x over previous
"""DeepSeekMoE layer on 8 Trainium2 NeuronCores.

Strategy: token-parallel (data-parallel) across the 8 cores. Each core
processes 512 of the 4096 tokens: RMSNorm -> router (top-2 of 8, exact
fp32) -> all 8 routed experts (dense, masked by combine weights) + shared
expert. Expert matmuls run in fp32r (TF32-like) at full PE rate with
activations kept feature-major (transposed) so every matmul contracts on
the partition dim without extra weight transposes; weights are pre-packed
on the host into the exact SBUF tile layout.

Output is produced transposed per core (D-major) and untransposed on host
during the gather step.
"""
import sys

sys.path.insert(0, "/opt/trn_rl_repo")

import numpy as np
import concourse.bass as bass
import concourse.mybir as mybir
from concourse.masks import make_identity
from concourse.tile import TileContext, ScopedClock
from concourse.bass_utils import run_bass_kernel_spmd

fp32 = mybir.dt.float32
fp32r = mybir.dt.float32r
fp16 = mybir.dt.float16
i32 = mybir.dt.int32

# dtype used for the expert matmuls (weights + activations).
#  - fp32r: TF32-like, ~1.4e-4 matmul rel err
#  - fp16: ~5e-4 rel err, but 2-byte: half the weight DMA and pipelined
#    weight loads on the PE
import os
MM_DT_NAME = os.environ.get("MOE_MM_DT", "fp16")
AF = mybir.ActivationFunctionType
ALU = mybir.AluOpType
AX = mybir.AxisListType

# problem dims (hardcoded per contract)
B, T, D, F, E, K = 4, 1024, 1024, 512, 8, 2
N_CORES = 8
N = B * T              # 4096 tokens
C = N // N_CORES       # 512 tokens per core
CT = C // 128          # token tiles per core (4)
DT = D // 128          # d tiles (8)
FT = F // 128          # f tiles (4)
GATE_MAX = 30.0
LIN_MIN, LIN_MAX = -100.0, 100.0
EPS_RMS = 1e-6

# Exact clip ops (min(g,30), clip(u,+-100)). The reference applies them;
# with the given weight scales the bounds are never active, but they are
# cheap, so keep them on for exactness.
APPLY_CLIPS = True

# ---------------------------------------------------------------------------
# Workaround: this container's walrus build only accepts ONE semaphore wait
# per instruction ("Too many sync wait commands"). Split excess waits onto
# same-engine NoOp/Drain instructions.
# ---------------------------------------------------------------------------
MAX_WAITS = 1


class PatchedTileContext(TileContext):
    def _drain_and_barrier(self, tick_clock, wait_clock):
        drain_inst = self.nc.sync.drain()
        wait_clock.add_sem_waits(
            drain_inst.ins, ScopedClock({None: tick_clock.global_clock})
        )
        si = drain_inst.ins.sync_info
        waits = list(si.on_wait) if si is not None else []
        if len(waits) > MAX_WAITS:
            drain_inst.ins.sync_info.on_wait.clear()
            drain_inst.ins.sync_info.on_wait.extend(waits[:MAX_WAITS])
            for i in range(MAX_WAITS, len(waits), MAX_WAITS):
                extra = self.nc.sync.drain()
                extra.ins.sync_info = mybir.SyncInfo(
                    on_wait=list(waits[i : i + MAX_WAITS]), on_update=[]
                )
        self.nc.all_engine_barrier()
        assert self.sems is not None
        popped = self.nc._tile_sem_poison_stack.pop()
        assert popped is self._sem_poison
        self.nc.clear_and_free_semaphores(list(self.sems.allocated().values()))
        self.nc.all_engine_barrier()


def fix_excess_waits(nc, max_waits=MAX_WAITS):
    n_fixed = 0
    counter = [0]
    for f in nc.m.functions:
        for bb in f.blocks:
            il = bb.instructions
            new_list = []
            for inst in il:
                si = getattr(inst, "sync_info", None)
                waits = list(si.on_wait) if si is not None else []
                if len(waits) > max_waits:
                    n_fixed += 1
                    keep = waits[:max_waits]
                    rest = waits[max_waits:]
                    si.on_wait.clear()
                    si.on_wait.extend(keep)
                    for i in range(0, len(rest), max_waits):
                        counter[0] += 1
                        nop = mybir.InstNoOp(
                            name=f"I-waitfix-{counter[0]}", ins=[], outs=[]
                        )
                        nop.engine = inst.engine
                        nop.sync_info = mybir.SyncInfo(
                            on_wait=list(rest[i : i + max_waits]), on_update=[]
                        )
                        new_list.append(nop)
                new_list.append(inst)
            if len(new_list) != len(il):
                il.clear()
                il.extend(new_list)
    return n_fixed


# ---------------------------------------------------------------------------
# Device program (SPMD; identical on all 8 cores)
# ---------------------------------------------------------------------------

def build_nc(repeat=1, const_weights=None):
    mmdt = {"fp32r": fp32r, "fp16": fp16}[MM_DT_NAME]
    nc = bass.Bass("TRN2", target_bir_lowering=False, debug=False,
                   num_devices=N_CORES)

    def _wtensor(name, shape, dtype):
        if const_weights is not None and name in const_weights:
            data = np.ascontiguousarray(const_weights[name])
            nc.inline_tensor(data, name=name)
            mls = nc.lookup_mls(name)
            mls.dtype = dtype
            from concourse.bass_types import DRamTensorHandle as _DH
            return _DH(name, list(data.shape), dtype).ap()
        return nc.dram_tensor(name, shape, dtype, kind="ExternalInput").ap()

    x_d = nc.dram_tensor("x", [CT, 128, D], fp32, kind="ExternalInput").ap()
    vis_d = nc.dram_tensor("vis", [CT, 128, 1], i32, kind="ExternalInput").ap()
    rmsw_d = _wtensor("rmsw", [D], fp32)
    rwT_d = _wtensor("rwT", [128, DT, E], fp32)
    bias0_d = _wtensor("bias0", [E], fp32)
    bias1_d = _wtensor("bias1", [E], fp32)
    # routed expert weights, pre-packed: (E, 128, nk, out_dim)
    wgT_d = _wtensor("wgT", [E, 128, DT, F], mmdt)
    wuT_d = _wtensor("wuT", [E, 128, DT, F], mmdt)
    wdT_d = _wtensor("wdT", [E, 128, FT, D], mmdt)
    shgT_d = _wtensor("shgT", [128, DT, F], mmdt)
    shuT_d = _wtensor("shuT", [128, DT, F], mmdt)
    shdT_d = _wtensor("shdT", [128, FT, D], mmdt)

    outT_d = nc.dram_tensor("outT", [DT, 128, C], fp32, kind="ExternalOutput").ap()
    comb_dram = nc.dram_tensor("comb_scratch", [E, C], fp32).ap()

    with PatchedTileContext(nc) as tc:
        with (
            tc.tile_pool(name="const", bufs=1) as const,
            tc.tile_pool(name="xin", bufs=2) as xin,
            tc.tile_pool(name="hbuf", bufs=2) as hbuf,
            tc.tile_pool(name="persist", bufs=1) as persist,
            tc.tile_pool(name="router", bufs=4) as router,
            tc.tile_pool(name="combp", bufs=1) as combp,
            tc.tile_pool(name="wpool", bufs=2) as wpool,
            tc.tile_pool(name="wupool", bufs=2) as wupool,
            tc.tile_pool(name="wdpool", bufs=2) as wdpool,
            tc.tile_pool(name="act", bufs=2) as actp,
            tc.tile_pool(name="a2pool", bufs=2) as a2pool,
            tc.tile_pool(name="pst", bufs=2, space="PSUM") as pst,
            tc.tile_pool(name="psgu", bufs=2, space="PSUM") as psgu,
            tc.tile_pool(name="psy", bufs=2, space="PSUM") as psy,
        ):
            ident = const.tile([128, 128], fp32)
            make_identity(nc, ident[:])
            eps_t = const.tile([128, 1], fp32)
            nc.vector.memset(eps_t[:], EPS_RMS)
            rmsw_bc = const.tile([128, D], fp32)
            nc.gpsimd.dma_start(
                out=rmsw_bc[:],
                in_=bass.AP(tensor=rmsw_d.tensor, offset=rmsw_d.offset,
                            ap=[[0, 128]] + list(rmsw_d.ap)),
            )
            base_bc = const.tile([128, E], fp32)
            nc.gpsimd.dma_start(
                out=base_bc[:],
                in_=bass.AP(tensor=bias0_d.tensor, offset=bias0_d.offset,
                            ap=[[0, 128]] + list(bias0_d.ap)),
            )
            delta_bc = const.tile([128, E], fp32)
            nc.gpsimd.dma_start(
                out=delta_bc[:],
                in_=bass.AP(tensor=bias1_d.tensor, offset=bias1_d.offset,
                            ap=[[0, 128]] + list(bias1_d.ap)),
            )
            rwT = const.tile([128, DT, E], fp32)
            nc.gpsimd.dma_start(out=rwT[:], in_=rwT_d[:])

            hT = persist.tile([128, DT, C], fp32)      # h transposed (exact)
            hTr = persist.tile([128, DT, C], mmdt)    # fp32r copy for experts
            comb_bc = persist.tile([128, E, C], fp32)  # combine weights bcast
            outT = persist.tile([128, DT, C], fp32)    # output accumulator

            for r in range(repeat):
                # ---- RMSNorm + transpose h
                for tt in range(CT):
                    xt = xin.tile([128, D], fp32)
                    nc.gpsimd.dma_start(out=xt[:], in_=x_d[tt])
                    sq = hbuf.tile([128, D], fp32, tag="h")
                    var = router.tile([128, 1], fp32)
                    nc.scalar.activation(sq[:], xt[:], AF.Square,
                                         accum_out=var[:])
                    s = router.tile([128, 1], fp32)
                    nc.scalar.activation(s[:], var[:], AF.Sqrt,
                                         scale=1.0 / D, bias=eps_t[:])
                    rstd = router.tile([128, 1], fp32)
                    nc.vector.reciprocal(rstd[:], s[:])
                    ht = hbuf.tile([128, D], fp32, tag="h")
                    nc.vector.scalar_tensor_tensor(
                        ht[:], xt[:], rstd[:], rmsw_bc[:],
                        op0=ALU.mult, op1=ALU.mult)
                    for dt in range(DT):
                        tp = pst.tile([128, 128], fp32, tag="ps")
                        nc.tensor.transpose(
                            tp[:], ht[:, dt * 128:(dt + 1) * 128], ident[:])
                        nc.scalar.copy(hT[:, dt, tt * 128:(tt + 1) * 128], tp[:])
                        nc.vector.tensor_copy(
                            hTr[:, dt, tt * 128:(tt + 1) * 128], tp[:])

                # ---- router: z, aff, top-2, comb (token-major, exact fp32)
                combT = combp.tile([128, C], fp32)
                for tt in range(CT):
                    zp = pst.tile([128, E], fp32, tag="ps")
                    for dt in range(DT):
                        nc.tensor.matmul(
                            zp[:], hT[:, dt, tt * 128:(tt + 1) * 128],
                            rwT[:, dt, :], start=(dt == 0), stop=(dt == DT - 1))
                    aff = router.tile([128, E], fp32)
                    nc.scalar.activation(aff[:], zp[:], AF.Sigmoid)
                    vist = router.tile([128, 1], i32)
                    nc.gpsimd.dma_start(out=vist[:], in_=vis_d[tt])
                    visf = router.tile([128, 1], fp32)
                    nc.vector.tensor_copy(visf[:], vist[:])
                    biased = router.tile([128, E], fp32)
                    nc.vector.tensor_add(biased[:], aff[:], base_bc[:])
                    nc.vector.scalar_tensor_tensor(
                        biased[:], delta_bc[:], visf[:], biased[:],
                        op0=ALU.mult, op1=ALU.add)
                    mx1 = router.tile([128, 1], fp32)
                    nc.vector.tensor_reduce(mx1[:], biased[:], AX.X, ALU.max)
                    msk1 = router.tile([128, E], fp32)
                    nc.vector.tensor_scalar(msk1[:], biased[:], mx1[:], None,
                                            ALU.is_ge)
                    biased2 = router.tile([128, E], fp32)
                    nc.vector.scalar_tensor_tensor(
                        biased2[:], msk1[:], -1e9, biased[:],
                        op0=ALU.mult, op1=ALU.add)
                    mx2 = router.tile([128, 1], fp32)
                    nc.vector.tensor_reduce(mx2[:], biased2[:], AX.X, ALU.max)
                    msk2 = router.tile([128, E], fp32)
                    nc.vector.tensor_scalar(msk2[:], biased2[:], mx2[:], None,
                                            ALU.is_ge)
                    tmp = router.tile([128, E], fp32)
                    g1 = router.tile([128, 1], fp32)
                    nc.vector.tensor_tensor(tmp[:], msk1[:], aff[:], ALU.mult)
                    nc.vector.tensor_reduce(g1[:], tmp[:], AX.X, ALU.add)
                    g2 = router.tile([128, 1], fp32)
                    nc.vector.tensor_tensor(tmp[:], msk2[:], aff[:], ALU.mult)
                    nc.vector.tensor_reduce(g2[:], tmp[:], AX.X, ALU.add)
                    den = router.tile([128, 1], fp32)
                    nc.vector.tensor_add(den[:], g1[:], g2[:])
                    nc.vector.tensor_scalar_add(den[:], den[:], 1e-12)
                    inv = router.tile([128, 1], fp32)
                    nc.vector.reciprocal(inv[:], den[:])
                    comb = router.tile([128, E], fp32)
                    nc.vector.tensor_scalar(comb[:], msk1[:], g1[:], None,
                                            ALU.mult)
                    nc.vector.scalar_tensor_tensor(
                        comb[:], msk2[:], g2[:], comb[:],
                        op0=ALU.mult, op1=ALU.add)
                    nc.vector.tensor_scalar(comb[:], comb[:], inv[:], None,
                                            ALU.mult)
                    # transpose comb (128, E) -> (E, 128) into combT
                    ctp = pst.tile([128, 128], fp32, tag="ps")
                    nc.tensor.transpose(ctp[:E + 0, :], comb[:], ident[:])
                    nc.scalar.copy(combT[:E, tt * 128:(tt + 1) * 128],
                                   ctp[:E, :])
                # broadcast each expert's combine row to 128 partitions
                # (bounce through DRAM: SBUF APs cannot partition-broadcast)
                nc.gpsimd.dma_start(out=comb_dram[:], in_=combT[:E, :])
                for e in range(E):
                    row = comb_dram[e]
                    nc.gpsimd.dma_start(
                        out=comb_bc[:, e, :],
                        in_=bass.AP(tensor=row.tensor, offset=row.offset,
                                    ap=[[0, 128]] + list(row.ap)),
                    )

                # ---- expert pipeline: shared first, then routed experts
                for si in range(E + 1):
                    if si == 0:
                        wg_src, wu_src, wd_src = shgT_d[:], shuT_d[:], shdT_d[:]
                    else:
                        e = si - 1
                        wg_src, wu_src, wd_src = wgT_d[e], wuT_d[e], wdT_d[e]
                    wg_t = wpool.tile([128, DT, F], mmdt, tag="wg")
                    nc.gpsimd.dma_start(out=wg_t[:], in_=wg_src)
                    wu_t = wupool.tile([128, DT, F], mmdt, tag="wu")
                    nc.gpsimd.dma_start(out=wu_t[:], in_=wu_src)
                    wd_t = wdpool.tile([128, FT, D], mmdt, tag="wd")
                    nc.gpsimd.dma_start(out=wd_t[:], in_=wd_src)

                    a2 = a2pool.tile([128, FT, C], mmdt)
                    for ft in range(FT):
                        gp = psgu.tile([128, C], fp32)
                        for dt in range(DT):
                            nc.tensor.matmul(
                                gp[:], wg_t[:, dt, ft * 128:(ft + 1) * 128],
                                hTr[:, dt, :], start=(dt == 0),
                                stop=(dt == DT - 1))
                        sg = actp.tile([128, C], fp32, tag="sg")
                        if APPLY_CLIPS:
                            gm = actp.tile([128, C], fp32, tag="tmp")
                            nc.vector.tensor_scalar_min(gm[:], gp[:], GATE_MAX)
                            nc.scalar.activation(sg[:], gm[:], AF.Silu)
                        else:
                            nc.scalar.activation(sg[:], gp[:], AF.Silu)
                        up = psgu.tile([128, C], fp32)
                        for dt in range(DT):
                            nc.tensor.matmul(
                                up[:], wu_t[:, dt, ft * 128:(ft + 1) * 128],
                                hTr[:, dt, :], start=(dt == 0),
                                stop=(dt == DT - 1))
                        if APPLY_CLIPS:
                            uc = actp.tile([128, C], fp32, tag="tmp")
                            nc.vector.tensor_scalar(uc[:], up[:], LIN_MAX,
                                                    LIN_MIN, ALU.min, ALU.max)
                            usrc = uc
                        else:
                            usrc = up
                        if si == 0:
                            nc.vector.tensor_tensor(a2[:, ft, :], sg[:],
                                                    usrc[:], ALU.mult)
                        else:
                            nc.vector.tensor_tensor(sg[:], sg[:], usrc[:],
                                                    ALU.mult)
                            nc.vector.tensor_tensor(a2[:, ft, :], sg[:],
                                                    comb_bc[:, si - 1, :],
                                                    ALU.mult)
                    for dt in range(DT):
                        yp = psy.tile([128, C], fp32)
                        for ft in range(FT):
                            nc.tensor.matmul(
                                yp[:], wd_t[:, ft, dt * 128:(dt + 1) * 128],
                                a2[:, ft, :], start=(ft == 0),
                                stop=(ft == FT - 1))
                        if si == 0:
                            nc.vector.tensor_copy(outT[:, dt, :], yp[:])
                        else:
                            nc.vector.tensor_tensor(outT[:, dt, :], yp[:],
                                                    outT[:, dt, :], ALU.add)

                # ---- store output (transposed layout; host untransposes)
                for dt in range(DT):
                    nc.gpsimd.dma_start(out=outT_d[dt], in_=outT[:, dt, :])

    fix_excess_waits(nc)
    return nc


def _pack(w):
    """(out_dim, in_dim) weight -> (128, in_dim/128, out_dim) lhsT tiles."""
    out_dim, in_dim = w.shape
    nk = in_dim // 128
    return np.ascontiguousarray(
        w.T.reshape(nk, 128, out_dim).transpose(1, 0, 2))


_CACHE = {}


def _cast(a):
    return a.astype(np.float16) if MM_DT_NAME == "fp16" else a


def _prep(x, is_visual, rms_w, router_w, aux_bias, mod_bias,
          sh_wg, sh_wu, sh_wd, wg, wu, wd):
    xf = np.ascontiguousarray(np.asarray(x, np.float32).reshape(N, D))
    visf = np.asarray(is_visual, np.int32).reshape(N, 1)
    shared = {
        "rmsw": np.asarray(rms_w, np.float32),
        "rwT": _pack(np.asarray(router_w, np.float32)),
        "bias0": np.asarray(aux_bias, np.float32)
        + np.asarray(mod_bias, np.float32)[0],
        "bias1": np.asarray(mod_bias, np.float32)[1]
        - np.asarray(mod_bias, np.float32)[0],
        "wgT": _cast(np.stack([_pack(np.asarray(wg, np.float32)[e]) for e in range(E)])),
        "wuT": _cast(np.stack([_pack(np.asarray(wu, np.float32)[e]) for e in range(E)])),
        "wdT": _cast(np.stack([_pack(np.asarray(wd, np.float32)[e]) for e in range(E)])),
        "shgT": _cast(_pack(np.asarray(sh_wg, np.float32))),
        "shuT": _cast(_pack(np.asarray(sh_wu, np.float32))),
        "shdT": _cast(_pack(np.asarray(sh_wd, np.float32))),
    }
    in_maps = []
    for c in range(N_CORES):
        m = dict(shared)
        m["x"] = xf[c * C:(c + 1) * C].reshape(CT, 128, D)
        m["vis"] = visf[c * C:(c + 1) * C].reshape(CT, 128, 1)
        in_maps.append(m)
    return in_maps


def kernel(**inputs):
    if "nc" not in _CACHE:
        _CACHE["nc"] = build_nc()
    nc = _CACHE["nc"]
    in_maps = _prep(**inputs)
    res = run_bass_kernel_spmd(nc, in_maps, list(range(N_CORES)))
    parts = []
    for c in range(N_CORES):
        outT = res.results[c]["outT"]  # (DT, 128, C)
        parts.append(outT.transpose(2, 0, 1).reshape(C, D))
    return np.concatenate(parts, axis=0).reshape(B, T, D).astype(np.float32)

